# revision 2
# baseline (speedup 1.0000x reference)
# Trainium2 Bass kernel for MixedChunkAttention.
#
# Sharding: 8 cores = 4 batches x 2-way tensor-parallel split of INNER
# (E=2048 -> 1024 per core). Each core processes one full batch (the
# cross-chunk kv cumsum stays core-local) and one half of the inner dim;
# the host sums the two partial outputs per batch and adds bout.
#
# Per-core dataflow (chunked over G=16 chunks of C=256 positions):
#   xT chunk [D, C] streamed to SBUF (host pre-transposes x)
#   xhT  = silu(Win^T @ xT + bin)                  [H, C]   (PE + ACT)
#   qqT/qkT/lqT/lkT = per-partition affines of xhT          (DVE)
#   lk_nat = transpose(lkT)                        [C, H]   (PE transpose)
#   attnT[m,n] = mask(relu^2(qkT_m^T @ qqT))       [C, C]   (PE + ACT + DVE)
#   v    = silu(x @ Wv + bv)   natural [C, E']              (PE + ACT)
#   gT   = silu(Wg^T @ xT + bg)          [E', C]            (PE + ACT)
#   vqlT[e,:] = S[:,e]^T @ lqT + sum_m v[m,e]^T @ attnT[m]  (PE, fused psum accum)
#   oT   = vqlT * gT                                        (DVE)
#   S   += lk_nat^T @ v   (kv state update, after vql read) (PE + DVE)
#   out[c,:] += oT_e^T @ Wout[e,:]  over e-tiles            (PE)
#
# All matmuls run in float32r (reduced-precision fp32, 4x the fp32 rate).

import numpy as np

B, S, D = 4, 4096, 1024
C, H, E = 256, 128, 2048
G = S // C            # 16 chunks
ELOC = E // 2         # per-core inner slice
T = D // 128          # 8 d-tiles
ET = ELOC // 128      # 8 e-tiles
NCORES = 8

_CACHE = {}


def _build_nc():
    import concourse.mybir as mybir
    import concourse.tile as tile
    from concourse import bacc
    from concourse.masks import make_identity

    F32, F32R = mybir.dt.float32, mybir.dt.float32r
    AF = mybir.ActivationFunctionType
    OP = mybir.AluOpType

    nc = bacc.Bacc()
    xT_d = nc.declare_dram_parameter("xT", [128, T, S], F32R, isOutput=False)
    wv_d = nc.declare_dram_parameter("wv", [128, T, ELOC], F32R, isOutput=False)
    wg_d = nc.declare_dram_parameter("wg", [128, T, ELOC], F32R, isOutput=False)
    win_d = nc.declare_dram_parameter("win", [128, T, H], F32R, isOutput=False)
    wout_d = nc.declare_dram_parameter("wout", [128, ET, D], F32R, isOutput=False)
    bv_d = nc.declare_dram_parameter("bv", [1, ELOC], F32R, isOutput=False)
    bgt_d = nc.declare_dram_parameter("bgt", [128, ET], F32, isOutput=False)
    aff_d = nc.declare_dram_parameter("aff", [128, 9], F32, isOutput=False)
    msk_d = nc.declare_dram_parameter("masks", [128, 512], F32, isOutput=False)
    one_d = nc.declare_dram_parameter("ones", [1, 128], F32R, isOutput=False)
    zs_d = nc.declare_dram_parameter("zeros", [128, ELOC], F32R, isOutput=False)
    out_d = nc.declare_dram_parameter("out", [S, D], F32, isOutput=True)

    with tile.TileContext(nc) as tc:
        with tc.tile_pool(name="wpool", bufs=1) as wpool, \
             tc.tile_pool(name="spool", bufs=1) as spool, \
             tc.tile_pool(name="xtp", bufs=2) as xtp, \
             tc.tile_pool(name="vp", bufs=2) as vp, \
             tc.tile_pool(name="gp", bufs=1) as gp, \
             tc.tile_pool(name="otp", bufs=2) as otp, \
             tc.tile_pool(name="osp", bufs=2) as osp, \
             tc.tile_pool(name="smallp", bufs=2) as smallp, \
             tc.tile_pool(name="ps512", bufs=4, space="PSUM") as ps512, \
             tc.tile_pool(name="ps256", bufs=4, space="PSUM") as ps256:

            # ---- persistent tiles ----
            wv_sb = wpool.tile([128, T, ELOC], F32R, name="wv_sb")
            wg_sb = wpool.tile([128, T, ELOC], F32R, name="wg_sb")
            win_sb = wpool.tile([128, T, H], F32R, name="win_sb")
            wout_sb = wpool.tile([128, ET, D], F32R, name="wout_sb")
            bv_sb = wpool.tile([1, ELOC], F32R, name="bv_sb")
            bgt_sb = wpool.tile([128, ET], F32, name="bgt_sb")
            aff_sb = wpool.tile([128, 9], F32, name="aff_sb")
            msk_sb = wpool.tile([128, 512], F32, name="msk_sb")
            one_sb = wpool.tile([1, 128], F32R, name="one_sb")
            ident = wpool.tile([128, 128], F32, name="ident")
            nc.sync.dma_start(out=wv_sb[:], in_=wv_d[:])
            nc.sync.dma_start(out=wg_sb[:], in_=wg_d[:])
            nc.sync.dma_start(out=win_sb[:], in_=win_d[:])
            nc.sync.dma_start(out=wout_sb[:], in_=wout_d[:])
            nc.sync.dma_start(out=bv_sb[:], in_=bv_d[:])
            nc.sync.dma_start(out=bgt_sb[:], in_=bgt_d[:])
            nc.sync.dma_start(out=aff_sb[:], in_=aff_d[:])
            nc.sync.dma_start(out=msk_sb[:], in_=msk_d[:])
            nc.sync.dma_start(out=one_sb[:], in_=one_d[:])
            make_identity(nc, ident)

            St = spool.tile([128, ELOC], F32R, name="St")
            nc.sync.dma_start(out=St[:], in_=zs_d[:])

            for g in range(G):
                # ---- load x^T chunk ----
                xt = xtp.tile([128, T, C], F32R, name="xt", tag="xt")
                nc.sync.dma_start(out=xt[:], in_=xT_d[:, :, g * C:(g + 1) * C])

                # ---- head projection xhT = silu(Win^T @ xT + bin) [H, C] ----
                xh_ps = ps256.tile([128, C], F32, name="xh_ps", tag="ps256")
                for t in range(T):
                    nc.tensor.matmul(xh_ps[:], win_sb[:, t, :], xt[:, t, :],
                                     start=(t == 0), stop=(t == T - 1))
                xh = smallp.tile([128, C], F32, name="xh", tag="xh")
                nc.scalar.activation(xh[:], xh_ps[:], AF.Silu,
                                     bias=aff_sb[:, 8:9])

                # ---- affines ----
                qqT = smallp.tile([128, C], F32R, name="qqT", tag="qqT")
                qkT = smallp.tile([128, C], F32R, name="qkT", tag="qkT")
                lqT = smallp.tile([128, C], F32R, name="lqT", tag="lqT")
                lkT = smallp.tile([128, C], F32, name="lkT", tag="lkT")
                nc.vector.tensor_scalar(out=qqT[:], in0=xh[:],
                                        scalar1=aff_sb[:, 0:1], scalar2=aff_sb[:, 1:2],
                                        op0=OP.mult, op1=OP.add)
                nc.vector.tensor_scalar(out=qkT[:], in0=xh[:],
                                        scalar1=aff_sb[:, 2:3], scalar2=aff_sb[:, 3:4],
                                        op0=OP.mult, op1=OP.add)
                nc.vector.tensor_scalar(out=lqT[:], in0=xh[:],
                                        scalar1=aff_sb[:, 4:5], scalar2=aff_sb[:, 5:6],
                                        op0=OP.mult, op1=OP.add)
                nc.vector.tensor_scalar(out=lkT[:], in0=xh[:],
                                        scalar1=aff_sb[:, 6:7], scalar2=aff_sb[:, 7:8],
                                        op0=OP.mult, op1=OP.add)

                # ---- lk natural via PE transpose ----
                lkn = smallp.tile([128, 2, H], F32R, name="lkn", tag="lkn")
                for ci in range(2):
                    tr_ps = ps256.tile([128, 128], F32, name="tr_ps", tag="ps256")
                    nc.tensor.transpose(tr_ps[:], lkT[:, ci * 128:(ci + 1) * 128],
                                        ident[:])
                    nc.vector.tensor_copy(lkn[:, ci, :], tr_ps[:])

                # ---- chunk attention attnT[m, n] ----
                attnT = smallp.tile([128, 2, C], F32R, name="attnT", tag="attnT")
                for mi in range(2):
                    at_ps = ps256.tile([128, C], F32, name="at_ps", tag="ps256")
                    nc.tensor.matmul(at_ps[:], qkT[:, mi * 128:(mi + 1) * 128],
                                     qqT[:], start=True, stop=True)
                    rt = smallp.tile([128, C], F32, name="rt", tag="rt")
                    nc.scalar.activation(rt[:], at_ps[:], AF.Relu, bias=0.0)
                    rtm = smallp.tile([128, C], F32, name="rtm", tag="rtm")
                    nc.vector.tensor_tensor(out=rtm[:], in0=rt[:],
                                            in1=msk_sb[:, mi * C:(mi + 1) * C],
                                            op=OP.mult)
                    nc.vector.tensor_tensor(out=attnT[:, mi, :], in0=rtm[:],
                                            in1=rtm[:], op=OP.mult)

                # ---- v natural [C, ELOC] ----
                v_sb = vp.tile([128, 2, ELOC], F32R, name="v_sb", tag="v_sb")
                for ci in range(2):
                    for e2 in range(2):
                        v_ps = ps512.tile([128, 512], F32, name="v_ps", tag="ps512")
                        nc.tensor.matmul(v_ps[:], one_sb[0:1, :],
                                         bv_sb[0:1, e2 * 512:(e2 + 1) * 512],
                                         start=True, stop=False)
                        for t in range(T):
                            nc.tensor.matmul(
                                v_ps[:], xt[:, t, ci * 128:(ci + 1) * 128],
                                wv_sb[:, t, e2 * 512:(e2 + 1) * 512],
                                start=False, stop=(t == T - 1))
                        nc.scalar.activation(
                            v_sb[:, ci, e2 * 512:(e2 + 1) * 512], v_ps[:], AF.Silu,
                            bias=0.0)

                # ---- gateT [ELOC, C] ----
                gT = gp.tile([128, ET, C], F32, name="gT", tag="gT")
                for et in range(ET):
                    g_ps = ps256.tile([128, C], F32, name="g_ps", tag="ps256")
                    for t in range(T):
                        nc.tensor.matmul(g_ps[:],
                                         wg_sb[:, t, et * 128:(et + 1) * 128],
                                         xt[:, t, :],
                                         start=(t == 0), stop=(t == T - 1))
                    nc.scalar.activation(gT[:, et, :], g_ps[:], AF.Silu,
                                         bias=bgt_sb[:, et:et + 1])

                # ---- v_lin + v_quad fused into one psum accum, then gate ----
                oT = otp.tile([128, ET, C], F32R, name="oT", tag="oT")
                for et in range(ET):
                    vql_ps = ps256.tile([128, C], F32, name="vql_ps", tag="ps256")
                    nc.tensor.matmul(vql_ps[:],
                                     St[:, et * 128:(et + 1) * 128], lqT[:],
                                     start=True, stop=False)
                    for mi in range(2):
                        nc.tensor.matmul(
                            vql_ps[:],
                            v_sb[:, mi, et * 128:(et + 1) * 128],
                            attnT[:, mi, :],
                            start=False, stop=(mi == 1))
                    nc.vector.tensor_tensor(out=oT[:, et, :], in0=gT[:, et, :],
                                            in1=vql_ps[:], op=OP.mult)

                # ---- kv state update S += lk_nat^T @ v (after vql read S) ----
                for e2 in range(2):
                    kv_ps = ps512.tile([128, 512], F32, name="kv_ps", tag="ps512")
                    for ci in range(2):
                        nc.tensor.matmul(kv_ps[:], lkn[:, ci, :],
                                         v_sb[:, ci, e2 * 512:(e2 + 1) * 512],
                                         start=(ci == 0), stop=(ci == 1))
                    nc.vector.tensor_tensor(out=St[:, e2 * 512:(e2 + 1) * 512],
                                            in0=St[:, e2 * 512:(e2 + 1) * 512],
                                            in1=kv_ps[:], op=OP.add)

                # ---- output projection out[c, :] = sum_e oT_e^T @ Wout ----
                ostage = osp.tile([128, 2, D], F32, name="ostage", tag="ostage")
                for ci in range(2):
                    for d2 in range(2):
                        o_ps = ps512.tile([128, 512], F32, name="o_ps", tag="ps512")
                        for et in range(ET):
                            nc.tensor.matmul(
                                o_ps[:],
                                oT[:, et, ci * 128:(ci + 1) * 128],
                                wout_sb[:, et, d2 * 512:(d2 + 1) * 512],
                                start=(et == 0), stop=(et == ET - 1))
                        nc.vector.tensor_copy(
                            ostage[:, ci, d2 * 512:(d2 + 1) * 512], o_ps[:])
                    nc.sync.dma_start(
                        out=out_d[g * C + ci * 128: g * C + (ci + 1) * 128, :],
                        in_=ostage[:, ci, :])

    nc.finalize()
    return nc


def _get_nc():
    if "nc" not in _CACHE:
        _CACHE["nc"] = _build_nc()
    return _CACHE["nc"]


def _prep_inputs(x, Wv, bv, Wg, bg, Win, bin_, Wout, bout,
                 g_qq, b_qq, g_qk, b_qk, g_lq, b_lq, g_lk, b_lk):
    f = np.float32
    scale = f(E) ** f(0.5)
    tri = np.triu(np.ones((128, 128), f))          # keep p <= col
    masks = np.zeros((128, 512), f)
    masks[:, 0:128] = tri
    masks[:, 128:256] = 1.0
    masks[:, 256:384] = 0.0
    masks[:, 384:512] = tri
    aff = np.stack([
        g_qq / scale, b_qq / scale, g_qk, b_qk,
        g_lq, b_lq, g_lk, b_lk, bin_], axis=1).astype(f)       # [128, 9]
    ones = np.ones((1, 128), f)
    zeros = np.zeros((128, ELOC), f)

    def dtile(w, n):          # [D, n] -> [128, T, n]
        return np.ascontiguousarray(w.reshape(T, 128, n).transpose(1, 0, 2))

    in_maps = []
    for core in range(NCORES):
        b, h = core // 2, core % 2
        sl = slice(h * ELOC, (h + 1) * ELOC)
        xT = np.ascontiguousarray(
            x[b].T.reshape(T, 128, S).transpose(1, 0, 2))      # [128, T, S]
        wout_l = np.ascontiguousarray(
            Wout[sl, :].reshape(ET, 128, D).transpose(1, 0, 2))  # [128, ET, D]
        in_maps.append({
            "xT": xT.astype(f),
            "wv": dtile(Wv[:, sl], ELOC).astype(f),
            "wg": dtile(Wg[:, sl], ELOC).astype(f),
            "win": dtile(Win, H).astype(f),
            "wout": wout_l.astype(f),
            "bv": bv[sl].reshape(1, ELOC).astype(f),
            "bgt": np.ascontiguousarray(bg[sl].reshape(ET, 128).T).astype(f),
            "aff": aff,
            "masks": masks,
            "ones": ones,
            "zeros": zeros,
        })
    return in_maps


def _run(inputs, trace=False, **trace_kw):
    from concourse.bass_utils import run_bass_kernel_spmd
    nc = _get_nc()
    in_maps = _prep_inputs(**inputs)
    res = run_bass_kernel_spmd(nc, in_maps, core_ids=list(range(NCORES)),
                               trace=trace, **trace_kw)
    bout = np.asarray(inputs["bout"], np.float32)
    out = np.zeros((B, S, D), np.float32)
    for core in range(NCORES):
        out[core // 2] += res.results[core]["out"]
    out += bout[None, None, :]
    return out, res


def kernel(**inputs) -> np.ndarray:
    inputs = {k: np.asarray(v) for k, v in inputs.items()}
    out, _ = _run(inputs)
    return out


# revision 3
# speedup vs baseline: 1.0101x; 1.0101x over previous
# Trainium2 Bass kernel for MixedChunkAttention.
#
# Sharding: 8 cores = 4 batches x 2-way tensor-parallel split of INNER
# (E=2048 -> 1024 per core). Each core processes one full batch (the
# cross-chunk kv cumsum stays core-local) and one half of the inner dim;
# the host sums the two partial outputs per batch and adds bout.
#
# Per-core dataflow (chunked over G=16 chunks of C=256 positions):
#   xT chunk [D, C] streamed to SBUF (host pre-transposes x)
#   xhT  = silu(Win^T @ xT + bin)                  [H, C]   (PE + ACT)
#   qqT/qkT/lqT/lkT = per-partition affines of xhT          (DVE)
#   lk_nat = transpose(lkT)                        [C, H]   (PE transpose)
#   attnT[m,n] = mask(relu^2(qkT_m^T @ qqT))       [C, C]   (PE + ACT + DVE)
#   v    = silu(x @ Wv + bv)   natural [C, E']              (PE + ACT)
#   gT   = silu(Wg^T @ xT + bg)          [E', C]            (PE + ACT)
#   vqlT[e,:] = S[:,e]^T @ lqT + sum_m v[m,e]^T @ attnT[m]  (PE, fused psum accum)
#   oT   = vqlT * gT                                        (DVE)
#   S   += lk_nat^T @ v   (kv state update, after vql read) (PE + DVE)
#   out[c,:] += oT_e^T @ Wout[e,:]  over e-tiles            (PE)
#
# All matmuls run in float32r (reduced-precision fp32, 4x the fp32 rate).

import numpy as np

B, S, D = 4, 4096, 1024
C, H, E = 256, 128, 2048
G = S // C            # 16 chunks
ELOC = E // 2         # per-core inner slice
T = D // 128          # 8 d-tiles
ET = ELOC // 128      # 8 e-tiles
NCORES = 8

_CACHE = {}


def _build_nc(n_chunks=G):
    import concourse.mybir as mybir
    import concourse.tile as tile
    from concourse import bacc
    from concourse.masks import make_identity

    F32, F32R = mybir.dt.float32, mybir.dt.float32r
    AF = mybir.ActivationFunctionType
    OP = mybir.AluOpType

    nc = bacc.Bacc()
    xT_d = nc.declare_dram_parameter("xT", [128, T, S], F32R, isOutput=False)
    wv_d = nc.declare_dram_parameter("wv", [128, T, ELOC], F32R, isOutput=False)
    wg_d = nc.declare_dram_parameter("wg", [128, T, ELOC], F32R, isOutput=False)
    win_d = nc.declare_dram_parameter("win", [128, T, H], F32R, isOutput=False)
    wout_d = nc.declare_dram_parameter("wout", [128, ET, D], F32R, isOutput=False)
    bv_d = nc.declare_dram_parameter("bv", [1, ELOC], F32R, isOutput=False)
    bgt_d = nc.declare_dram_parameter("bgt", [128, ET], F32, isOutput=False)
    aff_d = nc.declare_dram_parameter("aff", [128, 9], F32, isOutput=False)
    msk_d = nc.declare_dram_parameter("masks", [128, 512], F32, isOutput=False)
    one_d = nc.declare_dram_parameter("ones", [1, 128], F32R, isOutput=False)
    zs_d = nc.declare_dram_parameter("zeros", [128, ELOC], F32R, isOutput=False)
    out_d = nc.declare_dram_parameter("out", [S, D], F32, isOutput=True)

    with tile.TileContext(nc) as tc:
        with tc.tile_pool(name="wpool", bufs=1) as wpool, \
             tc.tile_pool(name="spool", bufs=1) as spool, \
             tc.tile_pool(name="xtp", bufs=2) as xtp, \
             tc.tile_pool(name="vp", bufs=2) as vp, \
             tc.tile_pool(name="gp", bufs=1) as gp, \
             tc.tile_pool(name="otp", bufs=2) as otp, \
             tc.tile_pool(name="osp", bufs=2) as osp, \
             tc.tile_pool(name="smallp", bufs=2) as smallp, \
             tc.tile_pool(name="ps512", bufs=4, space="PSUM") as ps512, \
             tc.tile_pool(name="ps256", bufs=4, space="PSUM") as ps256:

            # ---- persistent tiles ----
            wv_sb = wpool.tile([128, T, ELOC], F32R, name="wv_sb")
            wg_sb = wpool.tile([128, T, ELOC], F32R, name="wg_sb")
            win_sb = wpool.tile([128, T, H], F32R, name="win_sb")
            wout_sb = wpool.tile([128, ET, D], F32R, name="wout_sb")
            bv_sb = wpool.tile([1, ELOC], F32R, name="bv_sb")
            bgt_sb = wpool.tile([128, ET], F32, name="bgt_sb")
            aff_sb = wpool.tile([128, 9], F32, name="aff_sb")
            msk_sb = wpool.tile([128, 512], F32, name="msk_sb")
            one_sb = wpool.tile([1, 128], F32R, name="one_sb")
            ident = wpool.tile([128, 128], F32, name="ident")
            nc.sync.dma_start(out=wv_sb[:], in_=wv_d[:])
            nc.sync.dma_start(out=wg_sb[:], in_=wg_d[:])
            nc.sync.dma_start(out=win_sb[:], in_=win_d[:])
            nc.sync.dma_start(out=wout_sb[:], in_=wout_d[:])
            nc.sync.dma_start(out=bv_sb[:], in_=bv_d[:])
            nc.sync.dma_start(out=bgt_sb[:], in_=bgt_d[:])
            nc.sync.dma_start(out=aff_sb[:], in_=aff_d[:])
            nc.sync.dma_start(out=msk_sb[:], in_=msk_d[:])
            nc.sync.dma_start(out=one_sb[:], in_=one_d[:])
            make_identity(nc, ident)

            St = spool.tile([128, ELOC], F32R, name="St")
            nc.sync.dma_start(out=St[:], in_=zs_d[:])

            for g in range(n_chunks):
                # ---- load x^T chunk ----
                xt = xtp.tile([128, T, C], F32R, name="xt", tag="xt")
                nc.sync.dma_start(out=xt[:], in_=xT_d[:, :, g * C:(g + 1) * C])

                # ---- head projection xhT = silu(Win^T @ xT + bin) [H, C] ----
                xh_ps = ps256.tile([128, C], F32, name="xh_ps", tag="ps256")
                for t in range(T):
                    nc.tensor.matmul(xh_ps[:], win_sb[:, t, :], xt[:, t, :],
                                     start=(t == 0), stop=(t == T - 1))
                xh = smallp.tile([128, C], F32, name="xh", tag="xh")
                nc.scalar.activation(xh[:], xh_ps[:], AF.Silu,
                                     bias=aff_sb[:, 8:9])

                # ---- affines ----
                qqT = smallp.tile([128, C], F32R, name="qqT", tag="qqT")
                qkT = smallp.tile([128, C], F32R, name="qkT", tag="qkT")
                lqT = smallp.tile([128, C], F32R, name="lqT", tag="lqT")
                lkT = smallp.tile([128, C], F32, name="lkT", tag="lkT")
                nc.vector.tensor_scalar(out=qqT[:], in0=xh[:],
                                        scalar1=aff_sb[:, 0:1], scalar2=aff_sb[:, 1:2],
                                        op0=OP.mult, op1=OP.add)
                nc.vector.tensor_scalar(out=qkT[:], in0=xh[:],
                                        scalar1=aff_sb[:, 2:3], scalar2=aff_sb[:, 3:4],
                                        op0=OP.mult, op1=OP.add)
                nc.vector.tensor_scalar(out=lqT[:], in0=xh[:],
                                        scalar1=aff_sb[:, 4:5], scalar2=aff_sb[:, 5:6],
                                        op0=OP.mult, op1=OP.add)
                nc.vector.tensor_scalar(out=lkT[:], in0=xh[:],
                                        scalar1=aff_sb[:, 6:7], scalar2=aff_sb[:, 7:8],
                                        op0=OP.mult, op1=OP.add)

                # ---- lk natural via PE transpose ----
                lkn = smallp.tile([128, 2, H], F32R, name="lkn", tag="lkn")
                for ci in range(2):
                    tr_ps = ps256.tile([128, 128], F32, name="tr_ps", tag="ps256")
                    nc.tensor.transpose(tr_ps[:], lkT[:, ci * 128:(ci + 1) * 128],
                                        ident[:])
                    nc.vector.tensor_copy(lkn[:, ci, :], tr_ps[:])

                # ---- chunk attention attnT[m, n] ----
                attnT = smallp.tile([128, 2, C], F32R, name="attnT", tag="attnT")
                for mi in range(2):
                    at_ps = ps256.tile([128, C], F32, name="at_ps", tag="ps256")
                    nc.tensor.matmul(at_ps[:], qkT[:, mi * 128:(mi + 1) * 128],
                                     qqT[:], start=True, stop=True)
                    rt = smallp.tile([128, C], F32, name="rt", tag="rt")
                    nc.scalar.activation(rt[:], at_ps[:], AF.Relu, bias=0.0)
                    rtm = smallp.tile([128, C], F32, name="rtm", tag="rtm")
                    nc.vector.tensor_tensor(out=rtm[:], in0=rt[:],
                                            in1=msk_sb[:, mi * C:(mi + 1) * C],
                                            op=OP.mult)
                    nc.vector.tensor_tensor(out=attnT[:, mi, :], in0=rtm[:],
                                            in1=rtm[:], op=OP.mult)

                # ---- v natural [C, ELOC] ----
                v_sb = vp.tile([128, 2, ELOC], F32R, name="v_sb", tag="v_sb")
                for ci in range(2):
                    for e2 in range(2):
                        v_ps = ps512.tile([128, 512], F32, name="v_ps", tag="ps512")
                        nc.tensor.matmul(v_ps[:], one_sb[0:1, :],
                                         bv_sb[0:1, e2 * 512:(e2 + 1) * 512],
                                         start=True, stop=False)
                        for t in range(T):
                            nc.tensor.matmul(
                                v_ps[:], xt[:, t, ci * 128:(ci + 1) * 128],
                                wv_sb[:, t, e2 * 512:(e2 + 1) * 512],
                                start=False, stop=(t == T - 1))
                        nc.scalar.activation(
                            v_sb[:, ci, e2 * 512:(e2 + 1) * 512], v_ps[:], AF.Silu,
                            bias=0.0)

                # ---- gateT [ELOC, C] ----
                gT = gp.tile([128, ET, C], F32, name="gT", tag="gT")
                for et in range(ET):
                    g_ps = ps256.tile([128, C], F32, name="g_ps", tag="ps256")
                    for t in range(T):
                        nc.tensor.matmul(g_ps[:],
                                         wg_sb[:, t, et * 128:(et + 1) * 128],
                                         xt[:, t, :],
                                         start=(t == 0), stop=(t == T - 1))
                    nc.scalar.activation(gT[:, et, :], g_ps[:], AF.Silu,
                                         bias=bgt_sb[:, et:et + 1])

                # ---- v_lin + v_quad fused into one psum accum, then gate ----
                oT = otp.tile([128, ET, C], F32R, name="oT", tag="oT")
                for et in range(ET):
                    vql_ps = ps256.tile([128, C], F32, name="vql_ps", tag="ps256")
                    nc.tensor.matmul(vql_ps[:],
                                     St[:, et * 128:(et + 1) * 128], lqT[:],
                                     start=True, stop=False)
                    for mi in range(2):
                        nc.tensor.matmul(
                            vql_ps[:],
                            v_sb[:, mi, et * 128:(et + 1) * 128],
                            attnT[:, mi, :],
                            start=False, stop=(mi == 1))
                    nc.vector.tensor_tensor(out=oT[:, et, :], in0=gT[:, et, :],
                                            in1=vql_ps[:], op=OP.mult)

                # ---- kv state update S += lk_nat^T @ v (after vql read S) ----
                for e2 in range(2):
                    kv_ps = ps512.tile([128, 512], F32, name="kv_ps", tag="ps512")
                    for ci in range(2):
                        nc.tensor.matmul(kv_ps[:], lkn[:, ci, :],
                                         v_sb[:, ci, e2 * 512:(e2 + 1) * 512],
                                         start=(ci == 0), stop=(ci == 1))
                    nc.vector.tensor_tensor(out=St[:, e2 * 512:(e2 + 1) * 512],
                                            in0=St[:, e2 * 512:(e2 + 1) * 512],
                                            in1=kv_ps[:], op=OP.add)

                # ---- output projection out[c, :] = sum_e oT_e^T @ Wout ----
                ostage = osp.tile([128, 2, D], F32, name="ostage", tag="ostage")
                for ci in range(2):
                    for d2 in range(2):
                        o_ps = ps512.tile([128, 512], F32, name="o_ps", tag="ps512")
                        for et in range(ET):
                            nc.tensor.matmul(
                                o_ps[:],
                                oT[:, et, ci * 128:(ci + 1) * 128],
                                wout_sb[:, et, d2 * 512:(d2 + 1) * 512],
                                start=(et == 0), stop=(et == ET - 1))
                        nc.vector.tensor_copy(
                            ostage[:, ci, d2 * 512:(d2 + 1) * 512], o_ps[:])
                    nc.sync.dma_start(
                        out=out_d[g * C + ci * 128: g * C + (ci + 1) * 128, :],
                        in_=ostage[:, ci, :])

    nc.finalize()
    return nc


def _get_nc(n_chunks=G):
    key = ("nc", n_chunks)
    if key not in _CACHE:
        _CACHE[key] = _build_nc(n_chunks)
    return _CACHE[key]


def _prep_inputs(x, Wv, bv, Wg, bg, Win, bin_, Wout, bout,
                 g_qq, b_qq, g_qk, b_qk, g_lq, b_lq, g_lk, b_lk):
    f = np.float32
    scale = f(E) ** f(0.5)
    tri = np.triu(np.ones((128, 128), f))          # keep p <= col
    masks = np.zeros((128, 512), f)
    masks[:, 0:128] = tri
    masks[:, 128:256] = 1.0
    masks[:, 256:384] = 0.0
    masks[:, 384:512] = tri
    aff = np.stack([
        g_qq / scale, b_qq / scale, g_qk, b_qk,
        g_lq, b_lq, g_lk, b_lk, bin_], axis=1).astype(f)       # [128, 9]
    ones = np.ones((1, 128), f)
    zeros = np.zeros((128, ELOC), f)

    def dtile(w, n):          # [D, n] -> [128, T, n]
        return np.ascontiguousarray(w.reshape(T, 128, n).transpose(1, 0, 2))

    in_maps = []
    for core in range(NCORES):
        b, h = core // 2, core % 2
        sl = slice(h * ELOC, (h + 1) * ELOC)
        xT = np.ascontiguousarray(
            x[b].T.reshape(T, 128, S).transpose(1, 0, 2))      # [128, T, S]
        wout_l = np.ascontiguousarray(
            Wout[sl, :].reshape(ET, 128, D).transpose(1, 0, 2))  # [128, ET, D]
        in_maps.append({
            "xT": xT.astype(f),
            "wv": dtile(Wv[:, sl], ELOC).astype(f),
            "wg": dtile(Wg[:, sl], ELOC).astype(f),
            "win": dtile(Win, H).astype(f),
            "wout": wout_l.astype(f),
            "bv": bv[sl].reshape(1, ELOC).astype(f),
            "bgt": np.ascontiguousarray(bg[sl].reshape(ET, 128).T).astype(f),
            "aff": aff,
            "masks": masks,
            "ones": ones,
            "zeros": zeros,
        })
    return in_maps


def _run(inputs, trace=False, **trace_kw):
    from concourse.bass_utils import run_bass_kernel_spmd
    nc = _get_nc()
    in_maps = _prep_inputs(**inputs)
    res = run_bass_kernel_spmd(nc, in_maps, core_ids=list(range(NCORES)),
                               trace=trace, **trace_kw)
    bout = np.asarray(inputs["bout"], np.float32)
    out = np.zeros((B, S, D), np.float32)
    for core in range(NCORES):
        out[core // 2] += res.results[core]["out"]
    out += bout[None, None, :]
    return out, res


def kernel(**inputs) -> np.ndarray:
    inputs = {k: np.asarray(v) for k, v in inputs.items()}
    out, _ = _run(inputs)
    return out


# revision 6
# speedup vs baseline: 177.0552x; 175.2785x over previous
# Trainium2 Bass kernel for MixedChunkAttention.
#
# Sharding: 8 cores = 4 batches x 2-way tensor-parallel split of INNER
# (E=2048 -> 1024 per core). Each core processes one full batch (the
# cross-chunk kv cumsum stays core-local) and one half of the inner dim;
# the host sums the two partial outputs per batch and adds bout.
#
# Per-core dataflow (chunked over G=16 chunks of C=256 positions):
#   xT chunk [D, C] streamed to SBUF (host pre-transposes x)
#   xhT  = silu(Win^T @ xT + bin)                  [H, C]   (PE + ACT)
#   qqT/qkT/lqT/lkT = per-partition affines of xhT          (DVE)
#   lk_nat = transpose(lkT)                        [C, H]   (PE transpose)
#   attnT[m,n] = mask(relu^2(qkT_m^T @ qqT))       [C, C]   (PE + ACT + DVE)
#   v    = silu(x @ Wv + bv)   natural [C, E']              (PE + ACT)
#   gT   = silu(Wg^T @ xT + bg)          [E', C]            (PE + ACT)
#   vqlT[e,:] = S[:,e]^T @ lqT + sum_m v[m,e]^T @ attnT[m]  (PE, fused psum accum)
#   oT   = vqlT * gT                                        (DVE)
#   S   += lk_nat^T @ v   (kv state update, after vql read) (PE + DVE)
#   out[c,:] += oT_e^T @ Wout[e,:]  over e-tiles            (PE)
#
# All matmuls run in float32r (reduced-precision fp32, 4x the fp32 rate).

import numpy as np

B, S, D = 4, 4096, 1024
C, H, E = 256, 128, 2048
G = S // C            # 16 chunks
ELOC = E // 2         # per-core inner slice
T = D // 128          # 8 d-tiles
ET = ELOC // 128      # 8 e-tiles
NCORES = 8

_CACHE = {}


def _build_nc(n_chunks=G, reps=1):
    import concourse.mybir as mybir
    import concourse.tile as tile
    from concourse import bacc
    from concourse.masks import make_identity

    F32, F32R = mybir.dt.float32, mybir.dt.float32r
    AF = mybir.ActivationFunctionType
    OP = mybir.AluOpType

    nc = bacc.Bacc()
    xT_d = nc.declare_dram_parameter("xT", [128, T, S], F32R, isOutput=False)
    wv_d = nc.declare_dram_parameter("wv", [128, T, ELOC], F32R, isOutput=False)
    wg_d = nc.declare_dram_parameter("wg", [128, T, ELOC], F32R, isOutput=False)
    win_d = nc.declare_dram_parameter("win", [128, T, H], F32R, isOutput=False)
    wout_d = nc.declare_dram_parameter("wout", [128, ET, D], F32R, isOutput=False)
    bv_d = nc.declare_dram_parameter("bv", [1, ELOC], F32R, isOutput=False)
    bgt_d = nc.declare_dram_parameter("bgt", [128, ET], F32, isOutput=False)
    aff_d = nc.declare_dram_parameter("aff", [128, 9], F32, isOutput=False)
    msk_d = nc.declare_dram_parameter("masks", [128, 512], F32, isOutput=False)
    one_d = nc.declare_dram_parameter("ones", [1, 128], F32R, isOutput=False)
    zs_d = nc.declare_dram_parameter("zeros", [128, ELOC], F32R, isOutput=False)
    out_d = nc.declare_dram_parameter("out", [S, D], F32, isOutput=True)

    with tile.TileContext(nc) as tc:
        with tc.tile_pool(name="wpool", bufs=1) as wpool, \
             tc.tile_pool(name="spool", bufs=1) as spool, \
             tc.tile_pool(name="xtp", bufs=2) as xtp, \
             tc.tile_pool(name="vp", bufs=2) as vp, \
             tc.tile_pool(name="gp", bufs=1) as gp, \
             tc.tile_pool(name="otp", bufs=2) as otp, \
             tc.tile_pool(name="osp", bufs=2) as osp, \
             tc.tile_pool(name="smallp", bufs=2) as smallp, \
             tc.tile_pool(name="ps512", bufs=4, space="PSUM") as ps512, \
             tc.tile_pool(name="ps256", bufs=4, space="PSUM") as ps256:

            # ---- persistent tiles ----
            wv_sb = wpool.tile([128, T, ELOC], F32R, name="wv_sb")
            wg_sb = wpool.tile([128, T, ELOC], F32R, name="wg_sb")
            win_sb = wpool.tile([128, T, H], F32R, name="win_sb")
            wout_sb = wpool.tile([128, ET, D], F32R, name="wout_sb")
            bv_sb = wpool.tile([1, ELOC], F32R, name="bv_sb")
            bgt_sb = wpool.tile([128, ET], F32, name="bgt_sb")
            aff_sb = wpool.tile([128, 9], F32, name="aff_sb")
            msk_sb = wpool.tile([128, 512], F32, name="msk_sb")
            one_sb = wpool.tile([1, 128], F32R, name="one_sb")
            ident = wpool.tile([128, 128], F32, name="ident")
            nc.sync.dma_start(out=wv_sb[:], in_=wv_d[:])
            nc.sync.dma_start(out=wg_sb[:], in_=wg_d[:])
            nc.sync.dma_start(out=win_sb[:], in_=win_d[:])
            nc.sync.dma_start(out=wout_sb[:], in_=wout_d[:])
            nc.sync.dma_start(out=bv_sb[:], in_=bv_d[:])
            nc.sync.dma_start(out=bgt_sb[:], in_=bgt_d[:])
            nc.sync.dma_start(out=aff_sb[:], in_=aff_d[:])
            nc.sync.dma_start(out=msk_sb[:], in_=msk_d[:])
            nc.sync.dma_start(out=one_sb[:], in_=one_d[:])
            make_identity(nc, ident)

            St = spool.tile([128, ELOC], F32R, name="St")

            import contextlib
            rep_ctx = tc.For_i(0, reps) if reps > 1 else contextlib.nullcontext()
            with rep_ctx:
                nc.sync.dma_start(out=St[:], in_=zs_d[:])
                _chunk_body(nc, tc, n_chunks, locals())

    nc.finalize()
    return nc


def _chunk_body(nc, tc, n_chunks, env):
    import concourse.mybir as mybir
    F32, F32R = mybir.dt.float32, mybir.dt.float32r
    AF = mybir.ActivationFunctionType
    OP = mybir.AluOpType
    (xT_d, out_d) = (env["xT_d"], env["out_d"])
    (wv_sb, wg_sb, win_sb, wout_sb, bv_sb, bgt_sb, aff_sb, msk_sb, one_sb,
     ident, St) = (env[k] for k in
                   ["wv_sb", "wg_sb", "win_sb", "wout_sb", "bv_sb", "bgt_sb",
                    "aff_sb", "msk_sb", "one_sb", "ident", "St"])
    (xtp, vp, gp, otp, osp, smallp, ps512, ps256) = (
        env[k] for k in ["xtp", "vp", "gp", "otp", "osp", "smallp",
                         "ps512", "ps256"])
    if True:
        if True:
            for g in range(n_chunks):
                # ---- load x^T chunk ----
                xt = xtp.tile([128, T, C], F32R, name="xt", tag="xt")
                nc.sync.dma_start(out=xt[:], in_=xT_d[:, :, g * C:(g + 1) * C])

                # ---- head projection xhT = silu(Win^T @ xT + bin) [H, C] ----
                xh_ps = ps256.tile([128, C], F32, name="xh_ps", tag="ps256")
                for t in range(T):
                    nc.tensor.matmul(xh_ps[:], win_sb[:, t, :], xt[:, t, :],
                                     start=(t == 0), stop=(t == T - 1))
                xh = smallp.tile([128, C], F32, name="xh", tag="xh")
                nc.scalar.activation(xh[:], xh_ps[:], AF.Silu,
                                     bias=aff_sb[:, 8:9])

                # ---- affines ----
                qqT = smallp.tile([128, C], F32R, name="qqT", tag="qqT")
                qkT = smallp.tile([128, C], F32R, name="qkT", tag="qkT")
                lqT = smallp.tile([128, C], F32R, name="lqT", tag="lqT")
                lkT = smallp.tile([128, C], F32, name="lkT", tag="lkT")
                nc.vector.tensor_scalar(out=qqT[:], in0=xh[:],
                                        scalar1=aff_sb[:, 0:1], scalar2=aff_sb[:, 1:2],
                                        op0=OP.mult, op1=OP.add)
                nc.vector.tensor_scalar(out=qkT[:], in0=xh[:],
                                        scalar1=aff_sb[:, 2:3], scalar2=aff_sb[:, 3:4],
                                        op0=OP.mult, op1=OP.add)
                nc.vector.tensor_scalar(out=lqT[:], in0=xh[:],
                                        scalar1=aff_sb[:, 4:5], scalar2=aff_sb[:, 5:6],
                                        op0=OP.mult, op1=OP.add)
                nc.vector.tensor_scalar(out=lkT[:], in0=xh[:],
                                        scalar1=aff_sb[:, 6:7], scalar2=aff_sb[:, 7:8],
                                        op0=OP.mult, op1=OP.add)

                # ---- lk natural via PE transpose ----
                lkn = smallp.tile([128, 2, H], F32R, name="lkn", tag="lkn")
                for ci in range(2):
                    tr_ps = ps256.tile([128, 128], F32, name="tr_ps", tag="ps256")
                    nc.tensor.transpose(tr_ps[:], lkT[:, ci * 128:(ci + 1) * 128],
                                        ident[:])
                    nc.vector.tensor_copy(lkn[:, ci, :], tr_ps[:])

                # ---- chunk attention attnT[m, n] ----
                attnT = smallp.tile([128, 2, C], F32R, name="attnT", tag="attnT")
                for mi in range(2):
                    at_ps = ps256.tile([128, C], F32, name="at_ps", tag="ps256")
                    nc.tensor.matmul(at_ps[:], qkT[:, mi * 128:(mi + 1) * 128],
                                     qqT[:], start=True, stop=True)
                    rt = smallp.tile([128, C], F32, name="rt", tag="rt")
                    nc.scalar.activation(rt[:], at_ps[:], AF.Relu, bias=0.0)
                    rtm = smallp.tile([128, C], F32, name="rtm", tag="rtm")
                    nc.vector.tensor_tensor(out=rtm[:], in0=rt[:],
                                            in1=msk_sb[:, mi * C:(mi + 1) * C],
                                            op=OP.mult)
                    nc.vector.tensor_tensor(out=attnT[:, mi, :], in0=rtm[:],
                                            in1=rtm[:], op=OP.mult)

                # ---- v natural [C, ELOC] ----
                v_sb = vp.tile([128, 2, ELOC], F32R, name="v_sb", tag="v_sb")
                for ci in range(2):
                    for e2 in range(2):
                        v_ps = ps512.tile([128, 512], F32, name="v_ps", tag="ps512")
                        nc.tensor.matmul(v_ps[:], one_sb[0:1, :],
                                         bv_sb[0:1, e2 * 512:(e2 + 1) * 512],
                                         start=True, stop=False)
                        for t in range(T):
                            nc.tensor.matmul(
                                v_ps[:], xt[:, t, ci * 128:(ci + 1) * 128],
                                wv_sb[:, t, e2 * 512:(e2 + 1) * 512],
                                start=False, stop=(t == T - 1))
                        nc.scalar.activation(
                            v_sb[:, ci, e2 * 512:(e2 + 1) * 512], v_ps[:], AF.Silu,
                            bias=0.0)

                # ---- gateT [ELOC, C] ----
                gT = gp.tile([128, ET, C], F32, name="gT", tag="gT")
                for et in range(ET):
                    g_ps = ps256.tile([128, C], F32, name="g_ps", tag="ps256")
                    for t in range(T):
                        nc.tensor.matmul(g_ps[:],
                                         wg_sb[:, t, et * 128:(et + 1) * 128],
                                         xt[:, t, :],
                                         start=(t == 0), stop=(t == T - 1))
                    nc.scalar.activation(gT[:, et, :], g_ps[:], AF.Silu,
                                         bias=bgt_sb[:, et:et + 1])

                # ---- v_lin + v_quad fused into one psum accum, then gate ----
                oT = otp.tile([128, ET, C], F32R, name="oT", tag="oT")
                for et in range(ET):
                    vql_ps = ps256.tile([128, C], F32, name="vql_ps", tag="ps256")
                    nc.tensor.matmul(vql_ps[:],
                                     St[:, et * 128:(et + 1) * 128], lqT[:],
                                     start=True, stop=False)
                    for mi in range(2):
                        nc.tensor.matmul(
                            vql_ps[:],
                            v_sb[:, mi, et * 128:(et + 1) * 128],
                            attnT[:, mi, :],
                            start=False, stop=(mi == 1))
                    nc.vector.tensor_tensor(out=oT[:, et, :], in0=gT[:, et, :],
                                            in1=vql_ps[:], op=OP.mult)

                # ---- kv state update S += lk_nat^T @ v (after vql read S) ----
                for e2 in range(2):
                    kv_ps = ps512.tile([128, 512], F32, name="kv_ps", tag="ps512")
                    for ci in range(2):
                        nc.tensor.matmul(kv_ps[:], lkn[:, ci, :],
                                         v_sb[:, ci, e2 * 512:(e2 + 1) * 512],
                                         start=(ci == 0), stop=(ci == 1))
                    nc.vector.tensor_tensor(out=St[:, e2 * 512:(e2 + 1) * 512],
                                            in0=St[:, e2 * 512:(e2 + 1) * 512],
                                            in1=kv_ps[:], op=OP.add)

                # ---- output projection out[c, :] = sum_e oT_e^T @ Wout ----
                ostage = osp.tile([128, 2, D], F32, name="ostage", tag="ostage")
                for ci in range(2):
                    for d2 in range(2):
                        o_ps = ps512.tile([128, 512], F32, name="o_ps", tag="ps512")
                        for et in range(ET):
                            nc.tensor.matmul(
                                o_ps[:],
                                oT[:, et, ci * 128:(ci + 1) * 128],
                                wout_sb[:, et, d2 * 512:(d2 + 1) * 512],
                                start=(et == 0), stop=(et == ET - 1))
                        nc.vector.tensor_copy(
                            ostage[:, ci, d2 * 512:(d2 + 1) * 512], o_ps[:])
                    nc.sync.dma_start(
                        out=out_d[g * C + ci * 128: g * C + (ci + 1) * 128, :],
                        in_=ostage[:, ci, :])


def _get_nc(n_chunks=G, reps=1):
    key = ("nc", n_chunks, reps)
    if key not in _CACHE:
        _CACHE[key] = _build_nc(n_chunks, reps)
    return _CACHE[key]


def _prep_inputs(x, Wv, bv, Wg, bg, Win, bin_, Wout, bout,
                 g_qq, b_qq, g_qk, b_qk, g_lq, b_lq, g_lk, b_lk):
    f = np.float32
    scale = f(E) ** f(0.5)
    tri = np.triu(np.ones((128, 128), f))          # keep p <= col
    masks = np.zeros((128, 512), f)
    masks[:, 0:128] = tri
    masks[:, 128:256] = 1.0
    masks[:, 256:384] = 0.0
    masks[:, 384:512] = tri
    aff = np.stack([
        g_qq / scale, b_qq / scale, g_qk, b_qk,
        g_lq, b_lq, g_lk, b_lk, bin_], axis=1).astype(f)       # [128, 9]
    ones = np.ones((1, 128), f)
    zeros = np.zeros((128, ELOC), f)

    def dtile(w, n):          # [D, n] -> [128, T, n]
        return np.ascontiguousarray(w.reshape(T, 128, n).transpose(1, 0, 2))

    in_maps = []
    for core in range(NCORES):
        b, h = core // 2, core % 2
        sl = slice(h * ELOC, (h + 1) * ELOC)
        xT = np.ascontiguousarray(
            x[b].T.reshape(T, 128, S).transpose(1, 0, 2))      # [128, T, S]
        wout_l = np.ascontiguousarray(
            Wout[sl, :].reshape(ET, 128, D).transpose(1, 0, 2))  # [128, ET, D]
        in_maps.append({
            "xT": xT.astype(f),
            "wv": dtile(Wv[:, sl], ELOC).astype(f),
            "wg": dtile(Wg[:, sl], ELOC).astype(f),
            "win": dtile(Win, H).astype(f),
            "wout": wout_l.astype(f),
            "bv": bv[sl].reshape(1, ELOC).astype(f),
            "bgt": np.ascontiguousarray(bg[sl].reshape(ET, 128).T).astype(f),
            "aff": aff,
            "masks": masks,
            "ones": ones,
            "zeros": zeros,
        })
    return in_maps


def _run(inputs, trace=False, **trace_kw):
    from concourse.bass_utils import run_bass_kernel_spmd
    nc = _get_nc()
    in_maps = _prep_inputs(**inputs)
    res = run_bass_kernel_spmd(nc, in_maps, core_ids=list(range(NCORES)),
                               trace=trace, **trace_kw)
    bout = np.asarray(inputs["bout"], np.float32)
    out = np.zeros((B, S, D), np.float32)
    for core in range(NCORES):
        out[core // 2] += res.results[core]["out"]
    out += bout[None, None, :]
    return out, res


def kernel(**inputs) -> np.ndarray:
    inputs = {k: np.asarray(v) for k, v in inputs.items()}
    out, _ = _run(inputs)
    return out


# revision 10
# speedup vs baseline: 183.4444x; 1.0361x over previous
# Trainium2 Bass kernel for MixedChunkAttention.
#
# Sharding: 8 cores = 4 batches x 2-way tensor-parallel split of INNER
# (E=2048 -> 1024 per core). Each core processes one full batch (the
# cross-chunk kv cumsum stays core-local) and one half of the inner dim;
# the host sums the two partial outputs per batch and adds bout.
#
# Per-core dataflow (chunked over G=16 chunks of C=256 positions):
#   xT chunk [D, C] streamed to SBUF (host pre-transposes x)
#   xhT  = silu(Win^T @ xT + bin)                  [H, C]   (PE + ACT)
#   qqT/qkT/lqT/lkT = per-partition affines of xhT          (DVE)
#   lk_nat = transpose(lkT)                        [C, H]   (PE transpose)
#   attnT[m,n] = mask(relu^2(qkT_m^T @ qqT))       [C, C]   (PE + ACT + DVE)
#   v    = silu(x @ Wv + bv)   natural [C, E']              (PE + ACT)
#   gT   = silu(Wg^T @ xT + bg)          [E', C]            (PE + ACT)
#   vqlT[e,:] = S[:,e]^T @ lqT + sum_m v[m,e]^T @ attnT[m]  (PE, fused psum accum)
#   oT   = vqlT * gT                                        (DVE)
#   S   += lk_nat^T @ v   (kv state update, after vql read) (PE + DVE)
#   out[c,:] += oT_e^T @ Wout[e,:]  over e-tiles            (PE)
#
# All matmuls run in float32r (reduced-precision fp32, 4x the fp32 rate).

import numpy as np

B, S, D = 4, 4096, 1024
C, H, E = 256, 128, 2048
G = S // C            # 16 chunks
ELOC = E // 2         # per-core inner slice
T = D // 128          # 8 d-tiles
ET = ELOC // 128      # 8 e-tiles
NCORES = 8

_CACHE = {}
import os as _os
_PS512_BUFS = int(_os.environ.get("PS512_BUFS", "4"))
_PS256_BUFS = int(_os.environ.get("PS256_BUFS", "4"))


def _build_nc(n_chunks=G, reps=1, with_bv=True):
    import concourse.mybir as mybir
    import concourse.tile as tile
    from concourse import bacc
    from concourse.masks import make_identity

    F32, F32R = mybir.dt.float32, mybir.dt.float32r
    AF = mybir.ActivationFunctionType
    OP = mybir.AluOpType

    nc = bacc.Bacc()
    xT_d = nc.declare_dram_parameter("xT", [128, T, S], F32R, isOutput=False)
    wv_d = nc.declare_dram_parameter("wv", [128, T, ELOC], F32R, isOutput=False)
    wg_d = nc.declare_dram_parameter("wg", [128, T, ELOC], F32R, isOutput=False)
    win_d = nc.declare_dram_parameter("win", [128, T, H], F32R, isOutput=False)
    wout_d = nc.declare_dram_parameter("wout", [128, ET, D], F32R, isOutput=False)
    bv_d = nc.declare_dram_parameter("bv", [1, ELOC], F32R, isOutput=False)
    bgt_d = nc.declare_dram_parameter("bgt", [128, ET], F32, isOutput=False)
    aff_d = nc.declare_dram_parameter("aff", [128, 9], F32, isOutput=False)
    msk_d = nc.declare_dram_parameter("masks", [128, 512], F32, isOutput=False)
    one_d = nc.declare_dram_parameter("ones", [1, 128], F32R, isOutput=False)
    zs_d = nc.declare_dram_parameter("zeros", [128, ELOC], F32R, isOutput=False)
    out_d = nc.declare_dram_parameter("out", [S, D], F32, isOutput=True)

    with tile.TileContext(nc) as tc:
        with tc.tile_pool(name="wpool", bufs=1) as wpool, \
             tc.tile_pool(name="spool", bufs=1) as spool, \
             tc.tile_pool(name="xtp", bufs=2) as xtp, \
             tc.tile_pool(name="vp", bufs=1) as vp, \
             tc.tile_pool(name="gp", bufs=1) as gp, \
             tc.tile_pool(name="otp", bufs=1) as otp, \
             tc.tile_pool(name="osp", bufs=1) as osp, \
             tc.tile_pool(name="smallp", bufs=2) as smallp, \
             tc.tile_pool(name="ps512", bufs=_PS512_BUFS, space="PSUM") as ps512, \
             tc.tile_pool(name="ps256", bufs=_PS256_BUFS, space="PSUM") as ps256:

            # ---- persistent tiles ----
            wv_sb = wpool.tile([128, T, ELOC], F32R, name="wv_sb")
            wg_sb = wpool.tile([128, T, ELOC], F32R, name="wg_sb")
            win_sb = wpool.tile([128, T, H], F32R, name="win_sb")
            wout_sb = wpool.tile([128, ET, D], F32R, name="wout_sb")
            bv_sb = wpool.tile([1, ELOC], F32R, name="bv_sb")
            bgt_sb = wpool.tile([128, ET], F32, name="bgt_sb")
            aff_sb = wpool.tile([128, 9], F32, name="aff_sb")
            msk_sb = wpool.tile([128, 512], F32, name="msk_sb")
            one_sb = wpool.tile([1, 128], F32R, name="one_sb")
            ident = wpool.tile([128, 128], F32, name="ident")
            nc.sync.dma_start(out=wv_sb[:], in_=wv_d[:])
            nc.sync.dma_start(out=wg_sb[:], in_=wg_d[:])
            nc.sync.dma_start(out=win_sb[:], in_=win_d[:])
            nc.sync.dma_start(out=wout_sb[:], in_=wout_d[:])
            nc.sync.dma_start(out=bv_sb[:], in_=bv_d[:])
            nc.sync.dma_start(out=bgt_sb[:], in_=bgt_d[:])
            nc.sync.dma_start(out=aff_sb[:], in_=aff_d[:])
            nc.sync.dma_start(out=msk_sb[:], in_=msk_d[:])
            nc.sync.dma_start(out=one_sb[:], in_=one_d[:])
            make_identity(nc, ident)

            St = spool.tile([128, ELOC], F32R, name="St")

            import contextlib
            rep_ctx = tc.For_i(0, reps) if reps > 1 else contextlib.nullcontext()
            with rep_ctx:
                nc.sync.dma_start(out=St[:], in_=zs_d[:])
                _chunk_body(nc, tc, n_chunks, with_bv, locals())

    nc.finalize()
    return nc


def _chunk_body(nc, tc, n_chunks, with_bv, env):
    import concourse.mybir as mybir
    F32, F32R = mybir.dt.float32, mybir.dt.float32r
    AF = mybir.ActivationFunctionType
    OP = mybir.AluOpType
    (xT_d, out_d) = (env["xT_d"], env["out_d"])
    (wv_sb, wg_sb, win_sb, wout_sb, bv_sb, bgt_sb, aff_sb, msk_sb, one_sb,
     ident, St) = (env[k] for k in
                   ["wv_sb", "wg_sb", "win_sb", "wout_sb", "bv_sb", "bgt_sb",
                    "aff_sb", "msk_sb", "one_sb", "ident", "St"])
    (xtp, vp, gp, otp, osp, smallp, ps512, ps256) = (
        env[k] for k in ["xtp", "vp", "gp", "otp", "osp", "smallp",
                         "ps512", "ps256"])
    assert n_chunks % 2 == 0
    for gp_i in range(n_chunks // 2):
            # ---- load x^T chunk-pair, paired xh + gate projections ----
            xt = xtp.tile([128, T, 2 * C], F32R, name="xt", tag="xt")
            nc.sync.dma_start(out=xt[:],
                              in_=xT_d[:, :, gp_i * 2 * C:(gp_i + 1) * 2 * C])

            xh2 = smallp.tile([128, 2 * C], F32, name="xh2", tag="xh2", bufs=1)
            xh_ps = ps512.tile([128, 2 * C], F32, name="xh_ps", tag="ps512")
            for t in range(T):
                nc.tensor.matmul(xh_ps[:], win_sb[:, t, :], xt[:, t, :],
                                 start=(t == 0), stop=(t == T - 1))
            nc.scalar.activation(xh2[:], xh_ps[:], AF.Silu,
                                 bias=aff_sb[:, 8:9])

            gT2 = gp.tile([128, ET, 2 * C], F32, name="gT2", tag="gT2")
            for et in range(ET):
                g_ps = ps512.tile([128, 2 * C], F32, name="g_ps", tag="ps512")
                for t in range(T):
                    nc.tensor.matmul(g_ps[:],
                                     wg_sb[:, t, et * 128:(et + 1) * 128],
                                     xt[:, t, :],
                                     start=(t == 0), stop=(t == T - 1))
                nc.scalar.activation(gT2[:, et, :], g_ps[:], AF.Silu,
                                     bias=bgt_sb[:, et:et + 1])

            for gi in range(2):
                g = gp_i * 2 + gi
                co = gi * C                      # column offset into pair slabs
                xh = xh2[:, co:co + C]

                # ---- affines ----
                qqT = smallp.tile([128, C], F32R, name="qqT", tag="qqT")
                qkT = smallp.tile([128, C], F32R, name="qkT", tag="qkT")
                lqT = smallp.tile([128, C], F32R, name="lqT", tag="lqT")
                lkT = smallp.tile([128, C], F32, name="lkT", tag="lkT", bufs=1)
                nc.vector.tensor_scalar(out=qqT[:], in0=xh[:],
                                        scalar1=aff_sb[:, 0:1], scalar2=aff_sb[:, 1:2],
                                        op0=OP.mult, op1=OP.add)
                nc.vector.tensor_scalar(out=qkT[:], in0=xh[:],
                                        scalar1=aff_sb[:, 2:3], scalar2=aff_sb[:, 3:4],
                                        op0=OP.mult, op1=OP.add)
                nc.vector.tensor_scalar(out=lqT[:], in0=xh[:],
                                        scalar1=aff_sb[:, 4:5], scalar2=aff_sb[:, 5:6],
                                        op0=OP.mult, op1=OP.add)
                nc.vector.tensor_scalar(out=lkT[:], in0=xh[:],
                                        scalar1=aff_sb[:, 6:7], scalar2=aff_sb[:, 7:8],
                                        op0=OP.mult, op1=OP.add)

                # ---- lk natural via PE transpose ----
                lkn = smallp.tile([128, 2, H], F32R, name="lkn", tag="lkn")
                for ci in range(2):
                    tr_ps = ps256.tile([128, 128], F32, name="tr_ps", tag="ps256")
                    nc.tensor.transpose(tr_ps[:], lkT[:, ci * 128:(ci + 1) * 128],
                                        ident[:])
                    nc.vector.tensor_copy(lkn[:, ci, :], tr_ps[:])

                # ---- chunk attention attnT[m, n] ----
                attnT = smallp.tile([128, 2, C], F32R, name="attnT", tag="attnT")
                for mi in range(2):
                    at_ps = ps256.tile([128, C], F32, name="at_ps", tag="ps256")
                    nc.tensor.matmul(at_ps[:], qkT[:, mi * 128:(mi + 1) * 128],
                                     qqT[:], start=True, stop=True)
                    rt = smallp.tile([128, C], F32, name="rt", tag="rt")
                    nc.scalar.activation(rt[:], at_ps[:], AF.Relu, bias=0.0)
                    nc.vector.tensor_tensor(out=rt[:], in0=rt[:],
                                            in1=msk_sb[:, mi * C:(mi + 1) * C],
                                            op=OP.mult)
                    nc.vector.tensor_tensor(out=attnT[:, mi, :], in0=rt[:],
                                            in1=rt[:], op=OP.mult)

                # ---- v natural [C, ELOC] ----
                v_sb = vp.tile([128, 2, ELOC], F32R, name="v_sb", tag="v_sb")
                for ci in range(2):
                    for e2 in range(2):
                        v_ps = ps512.tile([128, 512], F32, name="v_ps", tag="ps512")
                        if with_bv:
                            nc.tensor.matmul(v_ps[:], one_sb[0:1, :],
                                             bv_sb[0:1, e2 * 512:(e2 + 1) * 512],
                                             start=True, stop=False)
                        for t in range(T):
                            nc.tensor.matmul(
                                v_ps[:], xt[:, t, co + ci * 128:co + (ci + 1) * 128],
                                wv_sb[:, t, e2 * 512:(e2 + 1) * 512],
                                start=(t == 0 and not with_bv),
                                stop=(t == T - 1))
                        nc.scalar.activation(
                            v_sb[:, ci, e2 * 512:(e2 + 1) * 512], v_ps[:], AF.Silu,
                            bias=0.0)

                # ---- v_lin + v_quad fused into one psum accum, then gate ----
                oT = otp.tile([128, ET, C], F32R, name="oT", tag="oT")
                for et in range(ET):
                    vql_ps = ps256.tile([128, C], F32, name="vql_ps", tag="ps256")
                    nc.tensor.matmul(vql_ps[:],
                                     St[:, et * 128:(et + 1) * 128], lqT[:],
                                     start=True, stop=False)
                    for mi in range(2):
                        nc.tensor.matmul(
                            vql_ps[:],
                            v_sb[:, mi, et * 128:(et + 1) * 128],
                            attnT[:, mi, :],
                            start=False, stop=(mi == 1))
                    nc.vector.tensor_tensor(out=oT[:, et, :],
                                            in0=gT2[:, et, co:co + C],
                                            in1=vql_ps[:], op=OP.mult)

                # ---- kv state update S += lk_nat^T @ v (after vql read S) ----
                for e2 in range(2):
                    kv_ps = ps512.tile([128, 512], F32, name="kv_ps", tag="ps512")
                    for ci in range(2):
                        nc.tensor.matmul(kv_ps[:], lkn[:, ci, :],
                                         v_sb[:, ci, e2 * 512:(e2 + 1) * 512],
                                         start=(ci == 0), stop=(ci == 1))
                    nc.vector.tensor_tensor(out=St[:, e2 * 512:(e2 + 1) * 512],
                                            in0=St[:, e2 * 512:(e2 + 1) * 512],
                                            in1=kv_ps[:], op=OP.add)

                # ---- output projection out[c, :] = sum_e oT_e^T @ Wout ----
                ostage = osp.tile([128, 2, D], F32, name="ostage", tag="ostage")
                for ci in range(2):
                    for d2 in range(2):
                        o_ps = ps512.tile([128, 512], F32, name="o_ps", tag="ps512")
                        for et in range(ET):
                            nc.tensor.matmul(
                                o_ps[:],
                                oT[:, et, ci * 128:(ci + 1) * 128],
                                wout_sb[:, et, d2 * 512:(d2 + 1) * 512],
                                start=(et == 0), stop=(et == ET - 1))
                        nc.vector.tensor_copy(
                            ostage[:, ci, d2 * 512:(d2 + 1) * 512], o_ps[:])
                    nc.sync.dma_start(
                        out=out_d[g * C + ci * 128: g * C + (ci + 1) * 128, :],
                        in_=ostage[:, ci, :])


def _get_nc(n_chunks=G, reps=1, with_bv=True):
    key = ("nc", n_chunks, reps, with_bv)
    if key not in _CACHE:
        _CACHE[key] = _build_nc(n_chunks, reps, with_bv)
    return _CACHE[key]


def _prep_inputs(x, Wv, bv, Wg, bg, Win, bin_, Wout, bout,
                 g_qq, b_qq, g_qk, b_qk, g_lq, b_lq, g_lk, b_lk):
    f = np.float32
    scale = f(E) ** f(0.5)
    tri = np.triu(np.ones((128, 128), f))          # keep p <= col
    masks = np.zeros((128, 512), f)
    masks[:, 0:128] = tri
    masks[:, 128:256] = 1.0
    masks[:, 256:384] = 0.0
    masks[:, 384:512] = tri
    aff = np.stack([
        g_qq / scale, b_qq / scale, g_qk, b_qk,
        g_lq, b_lq, g_lk, b_lk, bin_], axis=1).astype(f)       # [128, 9]
    ones = np.ones((1, 128), f)
    zeros = np.zeros((128, ELOC), f)

    def dtile(w, n):          # [D, n] -> [128, T, n]
        return np.ascontiguousarray(w.reshape(T, 128, n).transpose(1, 0, 2))

    in_maps = []
    for core in range(NCORES):
        b, h = core // 2, core % 2
        sl = slice(h * ELOC, (h + 1) * ELOC)
        xT = np.ascontiguousarray(
            x[b].T.reshape(T, 128, S).transpose(1, 0, 2))      # [128, T, S]
        wout_l = np.ascontiguousarray(
            Wout[sl, :].reshape(ET, 128, D).transpose(1, 0, 2))  # [128, ET, D]
        in_maps.append({
            "xT": xT.astype(f),
            "wv": dtile(Wv[:, sl], ELOC).astype(f),
            "wg": dtile(Wg[:, sl], ELOC).astype(f),
            "win": dtile(Win, H).astype(f),
            "wout": wout_l.astype(f),
            "bv": bv[sl].reshape(1, ELOC).astype(f),
            "bgt": np.ascontiguousarray(bg[sl].reshape(ET, 128).T).astype(f),
            "aff": aff,
            "masks": masks,
            "ones": ones,
            "zeros": zeros,
        })
    return in_maps


def _run(inputs, trace=False, reps=1, **trace_kw):
    from concourse.bass_utils import run_bass_kernel_spmd
    with_bv = bool(np.any(np.asarray(inputs["bv"])))
    nc = _get_nc(G, reps, with_bv)
    in_maps = _prep_inputs(**inputs)
    res = run_bass_kernel_spmd(nc, in_maps, core_ids=list(range(NCORES)),
                               trace=trace, **trace_kw)
    bout = np.asarray(inputs["bout"], np.float32)
    out = np.zeros((B, S, D), np.float32)
    for core in range(NCORES):
        out[core // 2] += res.results[core]["out"]
    out += bout[None, None, :]
    return out, res


def kernel(**inputs) -> np.ndarray:
    inputs = {k: np.asarray(v) for k, v in inputs.items()}
    out, _ = _run(inputs)
    return out
